# revision 17
# baseline (speedup 1.0000x reference)
"""Trainium2 Bass kernel for nn_Decoder (mean-pool L=16 + overlap-add step 8).

Math (per (b, c) slice, est = est_source[b, c] of shape [256, 4000]):
  A[g, f]      = (1/16) * sum_{l=0..15} est[16*g + l, f]          g in 0..15
  out[8*s + j] = A[j, s] + A[8+j, s-1]                            s in 0..4000
with A[., -1] = A[., 4000] = 0 at the edges.  Output length 8*4001 = 32008.

Layout strategy (8 cores, 4 slices each).  The overlap-add is folded into the
input on the host (z = low_half + high_half shifted one frame), halving device
HBM traffic vs loading both halves.  The group-of-16 row reduction stays on
device as matmuls against a block 1/16 weight matrix W [128, 8].

The host also PERMUTES the frame axis so both the matmul inputs and the DRAM
store are fully contiguous: frames s = 32*m + t (m in 0..127, t in 0..31) are
packed at column 128*t + m.  Matmul t then consumes the contiguous column
block [128t, 128t+128) as lhsT and produces psum[m, j] = y[256*m + 8*t + j],
i.e. the per-slice psum assembles as y viewed [128 partitions, 256] — each
partition holds a CONTIGUOUS 1 KiB run of the output, so each store is one
dense descriptor per partition (the previous layout scattered 32 B runs).

Walrus (pinned build) allows a single sync-wait per instruction: the warmup
matmul absorbs the W-load wait, each chunk's first matmul carries only that
chunk's load wait, psum/sbuf tiles are never reused (no second wait), and the
kernel tail drains with one wait_ge per proc (see _SingleWaitTileContext).
"""

import sys

if "/opt/trn_rl_repo" not in sys.path:
    sys.path.insert(0, "/opt/trn_rl_repo")

import numpy as np


def _install_ntff_hook():
    """Provide antenv.axon_hooks (absent in this image) so trace=True works.

    The boot-side installer (trn_agent_boot.trn_boot) skips hook setup when
    antenv.axon_hooks is missing; bass_utils then refuses to trace.  We
    register a lazy equivalent backed by the same ctypes NTFF driver.
    """
    import types
    try:
        import antenv
    except ImportError:
        return
    if "antenv.axon_hooks" in sys.modules:
        return
    mod = types.ModuleType("antenv.axon_hooks")
    _state = {}

    def set_axon_ntff_profile_hook(h):
        _state["h"] = h

    def get_axon_ntff_profile_hook():
        if "h" not in _state:
            try:
                from trn_agent_boot.trn_boot import _ntff_profile_via_ctypes
                _state["h"] = _ntff_profile_via_ctypes("/opt/axon/libaxon_pjrt.so")
            except Exception:
                _state["h"] = None
        return _state["h"]

    mod.set_axon_ntff_profile_hook = set_axon_ntff_profile_hook
    mod.get_axon_ntff_profile_hook = get_axon_ntff_profile_hook
    sys.modules["antenv.axon_hooks"] = mod
    antenv.axon_hooks = mod


_install_ntff_hook()

import concourse.bass as bass
import concourse.mybir as mybir
from concourse import tile
from concourse.bass_utils import run_bass_kernel_spmd


class _SingleWaitTileContext(tile.TileContext):
    """TileContext whose kernel-tail drain never carries multiple sem waits.

    The pinned walrus build rejects any instruction with more than one sync
    wait ("Too many sync wait commands").  Tile's default exit emits a single
    Drain waiting on every outstanding proc semaphore.  Instead, emit one
    wait_ge per proc on the SP sequencer (each a single-wait instruction),
    then a wait-free drain.
    """

    # proc indices >= _FIRST_DMA_PROC are DMA lanes whose semaphores advance
    # by 16 per op (one inc per SDMA engine) while the vector clock ticks 1.
    _FIRST_DMA_PROC = 11
    # Lanes whose completion is implied by program structure need no tail
    # wait: every LOAD lane was awaited by the compute that consumed it (so
    # its semaphore already hit target), and engine instruction retirement
    # is enforced by the all_engine_barrier below.  Only STORE lanes (whose
    # completion nothing else observes) must be waited out.  Set by
    # _build_nc to the store lanes' proc indices; None = wait everything.
    _WAIT_ONLY_PROCS: "set[int] | None" = None

    def _drain_and_barrier(self, tick_clock, wait_clock):
        nc = self.nc
        clock = tick_clock.global_clock  # bass_rust.VectorClock: 27 ints
        allocated = wait_clock.sems.allocated()
        for proc_idx, tick in enumerate(clock):
            if tick > 0 and proc_idx in allocated:
                if (self._WAIT_ONLY_PROCS is not None
                        and proc_idx not in self._WAIT_ONLY_PROCS):
                    continue
                val = tick * 16 if proc_idx >= self._FIRST_DMA_PROC else tick
                nc.sync.wait_ge(allocated[proc_idx], val)
        nc.sync.drain()
        nc.all_engine_barrier()
        popped = nc._tile_sem_poison_stack.pop()
        assert popped is self._sem_poison
        nc.clear_and_free_semaphores(list(self.sems.allocated().values()))
        nc.all_engine_barrier()


# Problem constants (hardcoded per spec)
B, C, D2, FRAMES = 16, 2, 256, 4000
L = 16
SUB = FRAMES + 1          # 4001 output subframes per slice
OUT_LEN = 8 * SUB         # 32008
N_CORES = 8
SLICES = (B * C) // N_CORES   # 4 slices per core
PADF = 4096               # padded frames per slice (32 tiles of 128)
NTILES = PADF // 128      # 32 matmuls per slice
# Pipeline chunks (slice, tile0, tile1).  Queue plan (one dma_start = one
# round-robin DGE lane; walrus allows ONE sync wait per instruction, so no
# lane may wrap: <=8 ops per DGE class):
#   SWDGE (5 ops):  W + chunk 0..3 stores.  Keeps W off the load-critical
#     scalar queue and SWDGE store traffic small enough that the gpsimd
#     drain hides under the load tail.
#   HWDGE (8 ops):  all 6 chunk loads (scalar queue, whose preamble ends
#     ~0.8us before sync's) + chunk 4's and 5's stores (sync queue, idle by
#     then; each gets its own DRAM tensor so no cross-queue WAW wait).
# The load path is AXI/HBM port-bound at ~295 GB/s; a second parallel HW
# load queue does NOT help (measured: it just splits the same bandwidth).
# Front chunks are coarse (fewer lanes and semaphores; the PE has ~8us of
# slack so a late start is free); the last two are tiny so the kernel tail
# after the final load byte is short.
CHUNK_LIST = [(0, 0, 32), (1, 0, 32), (2, 0, 32),
              (3, 0, 20), (3, 20, 28), (3, 28, 32)]

MM_DT_NP = np.float16     # device operand dtype: ~2e-4 rel err, halves HBM
OUT_DT_NP = np.float16    # store dtype (host upcasts): ~3e-4 rel err, halves
                          # store traffic and the SWDGE drain tail

_CACHE = {}


def _build_w() -> np.ndarray:
    w = np.zeros((128, 8), dtype=np.float32)
    for j in range(8):
        w[16 * j : 16 * j + 16, j] = 1.0 / L
    return w


def _build_nc() -> bass.Bass:
    mm_dt = mybir.dt.float16
    out_dt = mybir.dt.float16
    nchunks = len(CHUNK_LIST)
    nc = bass.Bass()
    # Host-packed input: z[i, d, 128*t + m] = zsum[i, d, 32*m + t] where
    # zsum = low_half + high_half shifted +1 frame, padded to 4096 frames.
    # Row length stays exactly PADF: a row stride of 8192 B keeps every DMA
    # run page-aligned (an 8-column W prefix cost ~20% load bandwidth).
    zd = nc.dram_tensor("z", [SLICES, 128, PADF], mm_dt, kind="ExternalInput")
    w = nc.dram_tensor("w", [128, 8], mm_dt, kind="ExternalInput")
    # Per-slice output, flat idx 256*m + 8*t + j; host trims and upcasts.
    y = nc.dram_tensor("y", [SLICES, 8 * PADF], out_dt, kind="ExternalOutput")
    # The last chunk stores into its own tensor: its sync-queue store must
    # not share a DRAM tensor with the SWDGE stores, else tile adds a
    # cross-queue WAW wait and the instruction exceeds walrus' 1-wait limit.
    y2 = nc.dram_tensor("y2", [128, 8 * (CHUNK_LIST[-1][2] - CHUNK_LIST[-1][1])],
                        out_dt, kind="ExternalOutput")
    y3 = nc.dram_tensor("y3", [128, 8 * (CHUNK_LIST[-2][2] - CHUNK_LIST[-2][1])],
                        out_dt, kind="ExternalOutput")

    # Tail waits: SWDGE emission order is W then stores c0..c3 -> store
    # lanes are DMASW1..4 (procs 12-15); HWDGE emission order is c0..c5
    # loads then the two sync stores -> lanes 6,7 (procs 25,26).
    _SingleWaitTileContext._WAIT_ONLY_PROCS = {12, 13, 14, 15, 25, 26}
    with _SingleWaitTileContext(nc) as tc:
        with (
            tc.tile_pool(name="wp", bufs=1) as wp,
            tc.tile_pool(name="zp", bufs=nchunks) as zp,
            tc.tile_pool(name="ob", bufs=nchunks) as obp,
            tc.tile_pool(name="ps", bufs=nchunks, space="PSUM") as psp,
        ):
            wt = wp.tile([128, 8], mm_dt)
            # W rides SWDGE: gpsimd dispatches it ~2us before the HW queues
            # spin up, so the warmup matmul never stalls the PE stream.
            nc.gpsimd.dma_start(out=wt[:], in_=w[:])

            ps_tiles = [psp.tile([128, 8 * (t1 - t0)], mybir.dt.float32,
                                 tag="ps", name=f"ps{n}")
                        for n, (_, t0, t1) in enumerate(CHUNK_LIST)]
            z_tiles = []
            # All loads are emitted first so each issuing queue dispatches
            # them as soon as its preamble retires.
            for n, (i, t0, t1) in enumerate(CHUNK_LIST):
                zt = zp.tile([128, 128 * (t1 - t0)], mm_dt, tag="zt",
                             name=f"zt{n}")
                nc.scalar.dma_start(out=zt[:], in_=zd[i, :, 128 * t0 : 128 * t1])
                z_tiles.append(zt)

            # Warmup matmul: absorbs the W-load wait on the PE so no real
            # matmul carries two sync waits (walrus limit).  It scribbles on
            # the LAST chunk's psum tile, whose real matmuls overwrite it
            # (start=True; same engine => ordered, no extra semaphore).
            nc.tensor.matmul(ps_tiles[-1][0:8, 0:8], wt[:], wt[:],
                             start=True, stop=True)

            for n, (i, t0, t1) in enumerate(CHUNK_LIST):
                tpc = t1 - t0
                zt = z_tiles[n]
                ob = obp.tile([128, 8 * tpc], out_dt, tag="ob", name=f"obt{n}")
                ps = ps_tiles[n]
                for q in range(tpc):
                    # psum[m, 8q+j] = sum_d zt[d, 128q+m] * W[d, j]
                    nc.tensor.matmul(
                        ps[:, 8 * q : 8 * q + 8],
                        zt[:, 128 * q : 128 * q + 128],
                        wt[:],
                        start=True, stop=True,
                    )
                # psum holds y[i] viewed [128, 256][:, 8*t0:8*t1]: partition
                # m covers flat y[256m + 8*t0 .. 256m + 8*t1) densely.
                nc.vector.tensor_copy(ob[:], ps[:])
                # Last (small) chunk stores on the by-then-idle sync HW
                # queue, overlapping the SWDGE drain; earlier chunks use
                # SWDGE so stores overlap loads without head-of-line
                # blocking them.
                if n == nchunks - 1:
                    # scalar queue: idle after the loads, and issuing here
                    # runs concurrently with y3's trigger on sync.
                    nc.scalar.dma_start(out=y2[:], in_=ob[:])
                elif n == nchunks - 2:
                    nc.sync.dma_start(out=y3[:], in_=ob[:])
                else:
                    nc.gpsimd.dma_start(
                        out=y[i].rearrange("(m n) -> m n", m=128)
                            [:, 8 * t0 : 8 * t1],
                        in_=ob[:],
                    )
    return nc


def _get_nc():
    if "nc" not in _CACHE:
        _CACHE["nc"] = _build_nc()
    return _CACHE["nc"]


def _prep_inputs(flat: np.ndarray) -> np.ndarray:
    """[S, 256, F] -> fp16 [S, 128, 4096], add-folded and column-permuted."""
    S = flat.shape[0]
    zs = np.zeros((S, 128, PADF), dtype=np.float32)
    zs[:, :, :FRAMES] += flat[:, :128, :]
    zs[:, :, 1 : FRAMES + 1] += flat[:, 128:, :]
    # permute: z[i, d, 128*t + m] = zs[i, d, 32*m + t]
    zp = zs.reshape(S, 128, 128, NTILES).transpose(0, 1, 3, 2)
    return np.ascontiguousarray(zp.reshape(S, 128, PADF), dtype=MM_DT_NP)


def kernel(est_source: np.ndarray, _trace: bool = False) -> np.ndarray:
    est = np.ascontiguousarray(np.asarray(est_source), dtype=np.float32)
    assert est.shape == (B, C, D2, FRAMES)
    flat = est.reshape(B * C, D2, FRAMES)
    z = _prep_inputs(flat)
    wmat = _build_w().astype(MM_DT_NP)

    nc = _get_nc()
    in_maps = [
        {"z": z[SLICES * k : SLICES * (k + 1)], "w": wmat}
        for k in range(N_CORES)
    ]
    res = run_bass_kernel_spmd(nc, in_maps, core_ids=list(range(N_CORES)),
                               trace=_trace)
    _CACHE["last_results"] = res
    outs = []
    for k in range(N_CORES):
        yk = res.results[k]["y"].reshape(SLICES, 128, 8 * NTILES).copy()
        for extra, (i_x, t0_x, t1_x) in (("y2", CHUNK_LIST[-1]),
                                         ("y3", CHUNK_LIST[-2])):
            yk[i_x, :, 8 * t0_x : 8 * t1_x] = res.results[k][extra]
        outs.append(yk.reshape(SLICES, 8 * PADF)[:, :OUT_LEN].astype(np.float32))
    return np.concatenate(outs, axis=0).reshape(B, C, OUT_LEN)


# revision 18
# speedup vs baseline: 1.0071x; 1.0071x over previous
"""Trainium2 Bass kernel for nn_Decoder (mean-pool L=16 + overlap-add step 8).

Math (per (b, c) slice, est = est_source[b, c] of shape [256, 4000]):
  A[g, f]      = (1/16) * sum_{l=0..15} est[16*g + l, f]          g in 0..15
  out[8*s + j] = A[j, s] + A[8+j, s-1]                            s in 0..4000
with A[., -1] = A[., 4000] = 0 at the edges.  Output length 8*4001 = 32008.

Layout strategy (8 cores, 4 slices each).  The overlap-add is folded into the
input on the host (z = low_half + high_half shifted one frame), halving device
HBM traffic vs loading both halves.  The group-of-16 row reduction stays on
device as matmuls against a block 1/16 weight matrix W [128, 8].

The host also PERMUTES the frame axis so both the matmul inputs and the DRAM
store are fully contiguous: frames s = 32*m + t (m in 0..127, t in 0..31) are
packed at column 128*t + m.  Matmul t then consumes the contiguous column
block [128t, 128t+128) as lhsT and produces psum[m, j] = y[256*m + 8*t + j],
i.e. the per-slice psum assembles as y viewed [128 partitions, 256] — each
partition holds a CONTIGUOUS 1 KiB run of the output, so each store is one
dense descriptor per partition (the previous layout scattered 32 B runs).

Walrus (pinned build) allows a single sync-wait per instruction: the warmup
matmul absorbs the W-load wait, each chunk's first matmul carries only that
chunk's load wait, psum/sbuf tiles are never reused (no second wait), and the
kernel tail drains with one wait_ge per proc (see _SingleWaitTileContext).
"""

import sys

if "/opt/trn_rl_repo" not in sys.path:
    sys.path.insert(0, "/opt/trn_rl_repo")

import numpy as np


def _install_ntff_hook():
    """Provide antenv.axon_hooks (absent in this image) so trace=True works.

    The boot-side installer (trn_agent_boot.trn_boot) skips hook setup when
    antenv.axon_hooks is missing; bass_utils then refuses to trace.  We
    register a lazy equivalent backed by the same ctypes NTFF driver.
    """
    import types
    try:
        import antenv
    except ImportError:
        return
    if "antenv.axon_hooks" in sys.modules:
        return
    mod = types.ModuleType("antenv.axon_hooks")
    _state = {}

    def set_axon_ntff_profile_hook(h):
        _state["h"] = h

    def get_axon_ntff_profile_hook():
        if "h" not in _state:
            try:
                from trn_agent_boot.trn_boot import _ntff_profile_via_ctypes
                _state["h"] = _ntff_profile_via_ctypes("/opt/axon/libaxon_pjrt.so")
            except Exception:
                _state["h"] = None
        return _state["h"]

    mod.set_axon_ntff_profile_hook = set_axon_ntff_profile_hook
    mod.get_axon_ntff_profile_hook = get_axon_ntff_profile_hook
    sys.modules["antenv.axon_hooks"] = mod
    antenv.axon_hooks = mod


_install_ntff_hook()

import concourse.bass as bass
import concourse.mybir as mybir
from concourse import tile
from concourse.bass_utils import run_bass_kernel_spmd


class _SingleWaitTileContext(tile.TileContext):
    """TileContext whose kernel-tail drain never carries multiple sem waits.

    The pinned walrus build rejects any instruction with more than one sync
    wait ("Too many sync wait commands").  Tile's default exit emits a single
    Drain waiting on every outstanding proc semaphore.  Instead, emit one
    wait_ge per proc on the SP sequencer (each a single-wait instruction),
    then a wait-free drain.
    """

    # proc indices >= _FIRST_DMA_PROC are DMA lanes whose semaphores advance
    # by 16 per op (one inc per SDMA engine) while the vector clock ticks 1.
    _FIRST_DMA_PROC = 11
    # Lanes whose completion is implied by program structure need no tail
    # wait: every LOAD lane was awaited by the compute that consumed it (so
    # its semaphore already hit target), and engine instruction retirement
    # is enforced by the all_engine_barrier below.  Only STORE lanes (whose
    # completion nothing else observes) must be waited out.  Set by
    # _build_nc to the store lanes' proc indices; None = wait everything.
    _WAIT_ONLY_PROCS: "set[int] | None" = None

    def _drain_and_barrier(self, tick_clock, wait_clock):
        nc = self.nc
        clock = tick_clock.global_clock  # bass_rust.VectorClock: 27 ints
        allocated = wait_clock.sems.allocated()
        for proc_idx, tick in enumerate(clock):
            if tick > 0 and proc_idx in allocated:
                if (self._WAIT_ONLY_PROCS is not None
                        and proc_idx not in self._WAIT_ONLY_PROCS):
                    continue
                val = tick * 16 if proc_idx >= self._FIRST_DMA_PROC else tick
                nc.sync.wait_ge(allocated[proc_idx], val)
        nc.sync.drain()
        nc.all_engine_barrier()
        popped = nc._tile_sem_poison_stack.pop()
        assert popped is self._sem_poison
        nc.clear_and_free_semaphores(list(self.sems.allocated().values()))
        nc.all_engine_barrier()


# Problem constants (hardcoded per spec)
B, C, D2, FRAMES = 16, 2, 256, 4000
L = 16
SUB = FRAMES + 1          # 4001 output subframes per slice
OUT_LEN = 8 * SUB         # 32008
N_CORES = 8
SLICES = (B * C) // N_CORES   # 4 slices per core
PADF = 4096               # padded frames per slice (32 tiles of 128)
NTILES = PADF // 128      # 32 matmuls per slice
# Pipeline chunks (slice, tile0, tile1).  Queue plan (one dma_start = one
# round-robin DGE lane; walrus allows ONE sync wait per instruction, so no
# lane may wrap: <=8 ops per DGE class):
#   SWDGE (4 ops):  W + chunk 0..2 stores.  Keeps W off the load-critical
#     scalar queue, and ends gpsimd's instruction stream early enough that
#     its fixed ~2us SWDGE drain hides completely under the load phase.
#   HWDGE (8 ops):  all 6 chunk loads (scalar queue, whose preamble ends
#     ~0.8us before sync's) + one merged chunk-3+4 store (sync queue: the
#     DVE casts retire in order, so waiting on cast 4's semaphore alone
#     covers both) + chunk 5's store (scalar queue, idle after the loads).
#     The late stores get their own DRAM tensors so no cross-queue WAW
#     wait appears.
# The load path is AXI/HBM port-bound at ~295 GB/s; a second parallel HW
# load queue does NOT help (measured: it just splits the same bandwidth).
# Front chunks are coarse (fewer lanes and semaphores; the PE has ~8us of
# slack so a late start is free); the last two are tiny so the kernel tail
# after the final load byte is short.
CHUNK_LIST = [(0, 0, 32), (1, 0, 32), (2, 0, 32),
              (3, 0, 20), (3, 20, 28), (3, 28, 32)]

MM_DT_NP = np.float16     # device operand dtype: ~2e-4 rel err, halves HBM
OUT_DT_NP = np.float16    # store dtype (host upcasts): ~3e-4 rel err, halves
                          # store traffic and the SWDGE drain tail

_CACHE = {}


def _build_w() -> np.ndarray:
    w = np.zeros((128, 8), dtype=np.float32)
    for j in range(8):
        w[16 * j : 16 * j + 16, j] = 1.0 / L
    return w


def _build_nc() -> bass.Bass:
    mm_dt = mybir.dt.float16
    out_dt = mybir.dt.float16
    nchunks = len(CHUNK_LIST)
    nc = bass.Bass()
    # Host-packed input: z[i, d, 128*t + m] = zsum[i, d, 32*m + t] where
    # zsum = low_half + high_half shifted +1 frame, padded to 4096 frames.
    # Row length stays exactly PADF: a row stride of 8192 B keeps every DMA
    # run page-aligned (an 8-column W prefix cost ~20% load bandwidth).
    zd = nc.dram_tensor("z", [SLICES, 128, PADF], mm_dt, kind="ExternalInput")
    w = nc.dram_tensor("w", [128, 8], mm_dt, kind="ExternalInput")
    # Per-slice output, flat idx 256*m + 8*t + j; host trims and upcasts.
    y = nc.dram_tensor("y", [SLICES, 8 * PADF], out_dt, kind="ExternalOutput")
    # The last chunk stores into its own tensor: its sync-queue store must
    # not share a DRAM tensor with the SWDGE stores, else tile adds a
    # cross-queue WAW wait and the instruction exceeds walrus' 1-wait limit.
    y2 = nc.dram_tensor("y2", [128, 8 * (CHUNK_LIST[-1][2] - CHUNK_LIST[-1][1])],
                        out_dt, kind="ExternalOutput")
    y3 = nc.dram_tensor("y3", [128, 8 * (CHUNK_LIST[-2][2] - CHUNK_LIST[-3][1])],
                        out_dt, kind="ExternalOutput")

    # Tail waits: SWDGE emission order is W then stores c0..c2 -> store
    # lanes are DMASW1..3 (procs 12-14); HWDGE emission order is c0..c5
    # loads then the merged and final stores -> lanes 6,7 (procs 25,26).
    _SingleWaitTileContext._WAIT_ONLY_PROCS = {12, 13, 14, 25, 26}
    with _SingleWaitTileContext(nc) as tc:
        with (
            tc.tile_pool(name="wp", bufs=1) as wp,
            tc.tile_pool(name="zp", bufs=nchunks) as zp,
            tc.tile_pool(name="ob", bufs=nchunks) as obp,
            tc.tile_pool(name="ps", bufs=nchunks, space="PSUM") as psp,
        ):
            wt = wp.tile([128, 8], mm_dt)
            # W rides SWDGE: gpsimd dispatches it ~2us before the HW queues
            # spin up, so the warmup matmul never stalls the PE stream.
            nc.gpsimd.dma_start(out=wt[:], in_=w[:])

            ps_tiles = [psp.tile([128, 8 * (t1 - t0)], mybir.dt.float32,
                                 tag="ps", name=f"ps{n}")
                        for n, (_, t0, t1) in enumerate(CHUNK_LIST)]
            z_tiles = []
            # All loads are emitted first so each issuing queue dispatches
            # them as soon as its preamble retires.
            for n, (i, t0, t1) in enumerate(CHUNK_LIST):
                zt = zp.tile([128, 128 * (t1 - t0)], mm_dt, tag="zt",
                             name=f"zt{n}")
                nc.scalar.dma_start(out=zt[:], in_=zd[i, :, 128 * t0 : 128 * t1])
                z_tiles.append(zt)

            # Warmup matmul: absorbs the W-load wait on the PE so no real
            # matmul carries two sync waits (walrus limit).  It scribbles on
            # the LAST chunk's psum tile, whose real matmuls overwrite it
            # (start=True; same engine => ordered, no extra semaphore).
            nc.tensor.matmul(ps_tiles[-1][0:8, 0:8], wt[:], wt[:],
                             start=True, stop=True)

            ob34 = None
            for n, (i, t0, t1) in enumerate(CHUNK_LIST):
                tpc = t1 - t0
                zt = z_tiles[n]
                if n == nchunks - 3:
                    # chunks -3 and -2 cast into one tile and store with a
                    # single DMA (their output columns are adjacent).
                    w34 = 8 * (CHUNK_LIST[-2][2] - t0)
                    ob34 = obp.tile([128, w34], out_dt, tag="ob", name="ob34")
                    ob = ob34[:, : 8 * tpc]
                elif n == nchunks - 2:
                    ob = ob34[:, 8 * (t0 - CHUNK_LIST[-3][1]) :]
                else:
                    ob = obp.tile([128, 8 * tpc], out_dt, tag="ob",
                                  name=f"obt{n}")
                ps = ps_tiles[n]
                for q in range(tpc):
                    # psum[m, 8q+j] = sum_d zt[d, 128q+m] * W[d, j]
                    nc.tensor.matmul(
                        ps[:, 8 * q : 8 * q + 8],
                        zt[:, 128 * q : 128 * q + 128],
                        wt[:],
                        start=True, stop=True,
                    )
                # psum holds y[i] viewed [128, 256][:, 8*t0:8*t1]: partition
                # m covers flat y[256m + 8*t0 .. 256m + 8*t1) densely.
                nc.vector.tensor_copy(ob[:], ps[:])
                # Last (small) chunk stores on the by-then-idle sync HW
                # queue, overlapping the SWDGE drain; earlier chunks use
                # SWDGE so stores overlap loads without head-of-line
                # blocking them.
                if n == nchunks - 1:
                    # scalar queue: idle after the loads, and issuing here
                    # runs concurrently with the merged store on sync.
                    nc.scalar.dma_start(out=y2[:], in_=ob[:])
                elif n == nchunks - 2:
                    nc.sync.dma_start(out=y3[:], in_=ob34[:])
                elif n == nchunks - 3:
                    pass  # stored together with chunk -2 above
                else:
                    nc.gpsimd.dma_start(
                        out=y[i].rearrange("(m n) -> m n", m=128)
                            [:, 8 * t0 : 8 * t1],
                        in_=ob[:],
                    )
    return nc


def _get_nc():
    if "nc" not in _CACHE:
        _CACHE["nc"] = _build_nc()
    return _CACHE["nc"]


def _prep_inputs(flat: np.ndarray) -> np.ndarray:
    """[S, 256, F] -> fp16 [S, 128, 4096], add-folded and column-permuted."""
    S = flat.shape[0]
    zs = np.zeros((S, 128, PADF), dtype=np.float32)
    zs[:, :, :FRAMES] += flat[:, :128, :]
    zs[:, :, 1 : FRAMES + 1] += flat[:, 128:, :]
    # permute: z[i, d, 128*t + m] = zs[i, d, 32*m + t]
    zp = zs.reshape(S, 128, 128, NTILES).transpose(0, 1, 3, 2)
    return np.ascontiguousarray(zp.reshape(S, 128, PADF), dtype=MM_DT_NP)


def kernel(est_source: np.ndarray, _trace: bool = False) -> np.ndarray:
    est = np.ascontiguousarray(np.asarray(est_source), dtype=np.float32)
    assert est.shape == (B, C, D2, FRAMES)
    flat = est.reshape(B * C, D2, FRAMES)
    z = _prep_inputs(flat)
    wmat = _build_w().astype(MM_DT_NP)

    nc = _get_nc()
    in_maps = [
        {"z": z[SLICES * k : SLICES * (k + 1)], "w": wmat}
        for k in range(N_CORES)
    ]
    res = run_bass_kernel_spmd(nc, in_maps, core_ids=list(range(N_CORES)),
                               trace=_trace)
    _CACHE["last_results"] = res
    outs = []
    for k in range(N_CORES):
        yk = res.results[k]["y"].reshape(SLICES, 128, 8 * NTILES).copy()
        i_x, t0_x, t1_x = CHUNK_LIST[-1]
        yk[i_x, :, 8 * t0_x : 8 * t1_x] = res.results[k]["y2"]
        i_x, t0_x = CHUNK_LIST[-3][0], CHUNK_LIST[-3][1]
        t1_x = CHUNK_LIST[-2][2]
        yk[i_x, :, 8 * t0_x : 8 * t1_x] = res.results[k]["y3"]
        outs.append(yk.reshape(SLICES, 8 * PADF)[:, :OUT_LEN].astype(np.float32))
    return np.concatenate(outs, axis=0).reshape(B, C, OUT_LEN)
